# revision 28
# baseline (speedup 1.0000x reference)
"""AttnBlock (GroupNorm + 1x1-conv QKV + spatial attention w/ softmax over
query-h + out-proj + residual) for Trainium2, 8 NeuronCores.

Sharding: core = 2*b + w_half  (4 samples x 2 halves of the w axis).
The softmax normalizes over the h index of the *query*, so for a fixed w
column the 64 h-values form one softmax group; splitting by w keeps every
group on one core.

v2 design (vs the fp32r baseline):
  - Query packing is h-major inside each 512-query block: q = qt*512 + h*8
    + ww (ww = w' - 8*qt). The softmax-normalize multiply then has its
    broadcast on a *middle* axis and packed bf16 innermost, which unlocks
    the DVE 2x_1p fast mode.
  - S = K^T Q runs as ONE fp8e4 DoubleRow matmul per (key-block, q-block):
    both operands are [ki, 2, *] with channel c = t*128 + ki. 2x PE rate.
  - exp reads a 2-bank [128, 1024] PSUM span in one ACT op, writes bf16.
  - d (softmax denominators, per (key, w')) via GpSimd half-fold (bf16
    tensor_add) + DVE strided segmented reduce; reciprocal on DVE (bf16).
  - attn = e * r broadcast-multiply on DVE at 2x (a few pairs on GpSimd).
  - O = V^T attn accumulates in bf16 (V^T tiles cast to bf16 at conv time).
  - GroupNorm is folded into the conv weights on device (as baseline).
"""

import os

import numpy as np

import concourse.bass as bass
import concourse.bacc as bacc
import concourse.mybir as mybir
import concourse.tile as tile
from concourse.bass_utils import run_bass_kernel_spmd

B, C, H, W = 4, 256, 64, 64
N = H * W            # 4096 keys
NH = N // 2          # 2048 queries per w-half
WH = W // 2          # 32 local w' values
GROUPS = 32
EPS = 1e-5
F32 = mybir.dt.float32
F32R = mybir.dt.float32r
BF16 = mybir.dt.bfloat16
FP8 = mybir.dt.float8e4
AF = mybir.ActivationFunctionType
ALU = mybir.AluOpType
AX = mybir.AxisListType
DR = mybir.MatmulPerfMode.DoubleRow


def _r(ap):
    return ap.bitcast(F32R)


def _bcast_mid(ap, n):
    """[p, ..., m] AP -> [p, ..., 0 x n, m]: broadcast over a new middle
    axis, keeping the packed innermost dim (preserves DVE 2x_1p)."""
    return bass.AP(tensor=ap.tensor, offset=ap.offset,
                   ap=[*ap.ap[:-1], [0, n], ap.ap[-1]])


def build_nc():
    nc = bacc.Bacc("TRN2", target_bir_lowering=False, debug=False)

    xf_d = nc.dram_tensor("xf", [C, N], BF16, kind="ExternalInput")
    xh_d = nc.dram_tensor("xh", [C, NH], F32, kind="ExternalInput")
    wT_d = {t: nc.dram_tensor(f"w{t}T", [C, C], F32, kind="ExternalInput")
            for t in "qkvo"}
    brow_d = {"v": nc.dram_tensor("bv_row", [1, C], F32, kind="ExternalInput")}
    bcol_d = {t: nc.dram_tensor(f"b{t}_col", [C, 1], F32, kind="ExternalInput")
              for t in "qko"}
    gamma_d = nc.dram_tensor("gamma_c", [C, 1], F32, kind="ExternalInput")
    beta_d = nc.dram_tensor("beta_c", [C, 1], F32, kind="ExternalInput")
    g1_d = nc.dram_tensor("G1", [C, GROUPS], F32, kind="ExternalInput")
    g2_d = nc.dram_tensor("G2", [GROUPS, C], F32, kind="ExternalInput")
    ones_d = nc.dram_tensor("ones_row", [1, 512], F32, kind="ExternalInput")
    out_d = nc.dram_tensor("out", [C, NH], F32, kind="ExternalOutput")

    with tile.TileContext(nc) as tc:
        with (
            nc.allow_low_precision(reason="bf16 softmax pipeline, 2e-2 gate"),
            tc.tile_pool(name="persist", bufs=1) as pp,
            tc.tile_pool(name="mm", bufs=3, space="PSUM") as pmm,
            tc.tile_pool(name="opsum", bufs=2, space="PSUM") as pop,
            tc.tile_pool(name="epool", bufs=2) as pe_pool,
            tc.tile_pool(name="upool", bufs=1) as pu_pool,
            tc.tile_pool(name="dpool", bufs=2) as pd_pool,
            tc.tile_pool(name="gnpool", bufs=2) as pgn_pool,
            tc.tile_pool(name="outpool", bufs=3) as pout,
        ):
            def ptile(shape, tag, dtype=F32):
                return pp.tile(shape, dtype, tag=tag, name=tag)

            def psum_t(tag_name):
                # [128, 1024] fp32 = 2 PSUM banks
                return pmm.tile([128, 1024], F32, tag="mm", name=tag_name)

            # ---------------- loads ----------------
            xf = []
            xh = []
            wT = {t: [] for t in "qkvo"}
            gam, bet, g1 = [], [], []
            for i in range(2):
                t = ptile([128, N], f"xf{i}", BF16)
                for ch in range(4):
                    nc.sync.dma_start(
                        out=t[:, 1024 * ch:1024 * (ch + 1)],
                        in_=xf_d[128 * i:128 * (i + 1),
                                 1024 * ch:1024 * (ch + 1)])
                xf.append(t)
            for i in range(2):
                t = ptile([128, NH], f"xh{i}", F32R)
                for ch in range(2):
                    nc.sync.dma_start(
                        out=t[:, 1024 * ch:1024 * (ch + 1)],
                        in_=xh_d[128 * i:128 * (i + 1),
                                 1024 * ch:1024 * (ch + 1)].bitcast(F32R))
                xh.append(t)
            for i in range(2):
                for w in "qkvo":
                    t = ptile([128, C], f"w{w}T{i}", F32R)
                    nc.sync.dma_start(out=t, in_=wT_d[w][128 * i:128 * (i + 1), :].bitcast(F32R))
                    wT[w].append(t)
                t = ptile([128, 1], f"gam{i}")
                nc.sync.dma_start(out=t, in_=gamma_d[128 * i:128 * (i + 1), :])
                gam.append(t)
                t = ptile([128, 1], f"bet{i}")
                nc.sync.dma_start(out=t, in_=beta_d[128 * i:128 * (i + 1), :])
                bet.append(t)
                t = ptile([128, GROUPS], f"g1_{i}")
                nc.sync.dma_start(out=t, in_=g1_d[128 * i:128 * (i + 1), :])
                g1.append(t)
            g2 = ptile([GROUPS, C], "g2")
            nc.sync.dma_start(out=g2, in_=g2_d[:, :])
            ones = ptile([1, 512], "ones", F32R)
            nc.sync.dma_start(out=ones, in_=ones_d[:, :].bitcast(F32R))
            brow = {}
            for w in "v":
                brow[w] = ptile([1, C], f"b{w}row", F32R)
                nc.sync.dma_start(out=brow[w], in_=brow_d[w][:, :].bitcast(F32R))
            bcol = {}
            for w in "qko":
                bcol[w] = []
                for i in range(2):
                    t = ptile([128, 1], f"b{w}col{i}")
                    nc.sync.dma_start(out=t, in_=bcol_d[w][128 * i:128 * (i + 1), :])
                    bcol[w].append(t)

            # ---------------- GroupNorm stats -> per-channel scale/shift ----
            NSUB = N // 512
            mstat = []
            for i in range(2):
                stats = pgn_pool.tile([128, NSUB, 6], F32, tag="gnstats",
                                     name=f"gnstats{i}")
                for s in range(NSUB):
                    nc.vector.bn_stats(out=stats[:, s, :],
                                       in_=xf[i][:, 512 * s:512 * (s + 1)])
                mv = pgn_pool.tile([128, 2], F32, tag="gnmv", name=f"gnmv{i}")
                nc.vector.bn_aggr(out=mv, in_=stats)
                ms = ptile([128, 2], f"mstat{i}")
                # ms = [mean_c, E[x^2]_c]
                nc.vector.tensor_mul(out=ms[:, 1:2], in0=mv[:, 0:1], in1=mv[:, 0:1])
                nc.vector.tensor_add(out=ms[:, 1:2], in0=ms[:, 1:2], in1=mv[:, 1:2])
                nc.vector.tensor_copy(out=ms[:, 0:1], in_=mv[:, 0:1])
                mstat.append(ms)

            pg_t = psum_t("pg")
            pg = pg_t[:GROUPS, :2]
            for i in range(2):
                nc.tensor.matmul(pg, lhsT=g1[i], rhs=mstat[i],
                                 start=(i == 0), stop=(i == 1))
            gstat = ptile([GROUPS, 2], "gstat")
            nc.vector.tensor_scalar_mul(out=gstat, in0=pg, scalar1=1.0 / 8.0)
            var32 = ptile([GROUPS, 1], "var32")
            nc.vector.tensor_mul(out=var32, in0=gstat[:, 0:1], in1=gstat[:, 0:1])
            nc.vector.tensor_sub(out=var32, in0=gstat[:, 1:2], in1=var32)
            std32 = ptile([GROUPS, 1], "std32")
            eps_t = ptile([GROUPS, 1], "eps_t")
            nc.vector.memset(eps_t, EPS)
            nc.scalar.activation(out=std32, in_=var32, func=AF.Sqrt, bias=eps_t)
            rstd = ptile([GROUPS, 1], "rstd")
            nc.vector.reciprocal(out=rstd, in_=std32)

            grstat = ptile([GROUPS, 2], "grstat")
            nc.vector.tensor_copy(out=grstat[:, 0:1], in_=gstat[:, 0:1])
            nc.vector.tensor_copy(out=grstat[:, 1:2], in_=rstd)

            sc, sh = [], []
            for i in range(2):
                pc_t = psum_t(f"pc{i}")
                pc = pc_t[:128, :2]
                nc.tensor.matmul(pc, lhsT=g2[:, 128 * i:128 * (i + 1)],
                                 rhs=grstat, start=True, stop=True)
                s = ptile([128, 1], f"sc{i}")
                nc.vector.tensor_mul(out=s, in0=pc[:, 1:2], in1=gam[i])
                sc.append(s)
                h = ptile([128, 1], f"sh{i}", F32R)
                nc.vector.tensor_mul(out=h, in0=pc[:, 0:1], in1=s)
                nc.vector.tensor_sub(out=h, in0=bet[i], in1=h)
                sh.append(h)

            # effective v bias as a row (per-free-column bias for V^T)
            beffr = {}
            for w in "v":
                rp_t = psum_t(f"br{w}")
                rp = rp_t[:1, :C]
                for i in range(2):
                    nc.tensor.matmul(rp, lhsT=sh[i], rhs=wT[w][i],
                                     start=(i == 0), stop=(i == 1))
                bt = ptile([1, C], f"beff{w}", F32R)
                nc.vector.tensor_add(out=bt, in0=rp, in1=brow[w])
                beffr[w] = bt
            # effective q,k biases as columns (per-partition bias for ACT fuse)
            beffc = {}
            for w in "qk":
                beffc[w] = []
                for j in range(2):
                    bp_t = psum_t(f"bc{w}{j}")
                    bp = bp_t[:128, :1]
                    for i in range(2):
                        nc.tensor.matmul(bp,
                                         lhsT=wT[w][i][:, 128 * j:128 * (j + 1)].bitcast(F32),
                                         rhs=sh[i].bitcast(F32),
                                         start=(i == 0), stop=(i == 1))
                    t = ptile([128, 1], f"beffc{w}{j}")
                    nc.vector.tensor_add(out=t, in0=bp, in1=bcol[w][j])
                    beffc[w].append(t)

            # fold GN scale into conv weights: q in place (f32r),
            # k/v as scaled bf16 copies (convs run bf16 against bf16 xf)
            for i in range(2):
                nc.vector.tensor_scalar_mul(out=wT["q"][i], in0=wT["q"][i],
                                            scalar1=sc[i])
            w_bf = {}
            for w in "kv":
                w_bf[w] = []
                for i in range(2):
                    t = ptile([128, C], f"w{w}bf{i}", BF16)
                    nc.scalar.activation(out=t, in_=wT[w][i].bitcast(F32),
                                         func=AF.Identity, scale=sc[i])
                    w_bf[w].append(t)
            ones_bf = ptile([1, 128], "ones_bf", BF16)
            nc.vector.memset(ones_bf, 1.0)
            beffr_bf = ptile([1, C], "beffr_bf", BF16)
            nc.vector.tensor_copy(out=beffr_bf, in_=beffr["v"].bitcast(F32))

            # ---------------- convs: K, Q (fp8 DoubleRow layout), V^T (bf16)
            # k8/q8 layout [ki, t, col]: channel c = t*128 + ki
            k8 = ptile([128, 2, N], "k8", FP8)
            q8 = ptile([128, 2, NH], "q8", FP8)
            for s2 in range(N // 1024):
                for j in range(2):
                    kp = psum_t(f"kp{j}_{s2}")
                    for half in range(2):
                        cols = slice(1024 * s2 + 512 * half,
                                     1024 * s2 + 512 * (half + 1))
                        for i in range(2):
                            nc.tensor.matmul(
                                kp[:, 512 * half:512 * (half + 1)],
                                lhsT=w_bf["k"][i][:, 128 * j:128 * (j + 1)],
                                rhs=xf[i][:, cols],
                                start=(i == 0), stop=(i == 1))
                    nc.scalar.activation(
                        out=k8[:, j, 1024 * s2:1024 * (s2 + 1)],
                        in_=kp, func=AF.Identity, bias=beffc["k"][j])
            for s2 in range(NH // 1024):
                for j in range(2):
                    qp = psum_t(f"qp{j}_{s2}")
                    for half in range(2):
                        cols = slice(1024 * s2 + 512 * half,
                                     1024 * s2 + 512 * (half + 1))
                        for i in range(2):
                            nc.tensor.matmul(
                                qp[:, 512 * half:512 * (half + 1)],
                                lhsT=_r(wT["q"][i][:, 128 * j:128 * (j + 1)]),
                                rhs=_r(xh[i][:, cols]),
                                start=(i == 0), stop=(i == 1))
                    nc.scalar.activation(
                        out=q8[:, j, 1024 * s2:1024 * (s2 + 1)],
                        in_=qp, func=AF.Identity, bias=beffc["q"][j])

            vT = [ptile([128, C], f"vT{rt}", BF16) for rt in range(N // 128)]

            def emit_vt(rt):
                vp_t = psum_t(f"vp{rt}")
                vp = vp_t[:, :C]
                for i in range(2):
                    nc.tensor.matmul(vp,
                                     lhsT=xf[i][:, 128 * rt:128 * (rt + 1)],
                                     rhs=w_bf["v"][i],
                                     start=(i == 0), stop=False)
                nc.tensor.matmul(vp, lhsT=ones_bf, rhs=beffr_bf,
                                 start=False, stop=True)
                t = vT[rt]
                if rt % 4 == 0:
                    nc.scalar.activation(out=t, in_=vp, func=AF.Identity)
                else:
                    nc.vector.tensor_copy(out=t, in_=vp)

            # bf16 copy of out-proj weights (O path runs bf16)
            wo_bf = []
            for i in range(2):
                t = ptile([128, C], f"wo_bf{i}", BF16)
                nc.vector.tensor_copy(out=t, in_=wT["o"][i].bitcast(F32))
                wo_bf.append(t)

            o_sb = [ptile([128, NH], "o_sb0", BF16), ptile([128, NH], "o_sb1", BF16)]

            kstage = os.environ.get("KSTAGE", "full")
            if kstage == "conv":
                for ct in range(2):
                    dbg = pout.tile([128, NH], F32, tag="dbg", name=f"dbg{ct}",
                                    bufs=2)
                    nc.vector.tensor_copy(out=dbg, in_=k8[:, ct, :NH])
                    nc.sync.dma_start(out=out_d[128 * ct:128 * (ct + 1), :],
                                      in_=dbg)

            # ---------------- attention (qt-phased, SW-pipelined) ----------
            # e_mega layout: idx = pair*1024 + t*512 + h*8 + ww
            # u_mega layout: idx = pair*512 + t*256 + ww*32 + h'
            # d/r layout:    idx = pair*16 + t*8 + ww
            NPAIR = N // 256     # 16 pairs of 128-key blocks
            NGP = 12             # pairs folded on GpSimd; rest on DVE
            NCH = 4              # pairs per reduce chunk
            e_ms = {}

            def emit_produce(qt):
                e_m = pe_pool.tile([128, NPAIR * 1024], BF16, tag="emega",
                                   name=f"em{qt}")
                e_ms[qt] = e_m
                for p in range(NPAIR):
                    emit_produce_pair(qt, p)

            def emit_produce_pair(qt, p):
                if p == 0 and qt not in e_ms:
                    e_ms[qt] = pe_pool.tile([128, NPAIR * 1024], BF16,
                                            tag="emega", name=f"em{qt}")
                e_m = e_ms[qt]
                qcols = slice(512 * qt, 512 * (qt + 1))
                sp = psum_t(f"sp{qt}_{p}")
                for t01 in range(2):
                    kb = 256 * p + 128 * t01
                    nc.tensor.matmul(
                        sp[:, 512 * t01:512 * (t01 + 1)],
                        lhsT=k8[:, :, kb:kb + 128],
                        rhs=q8[:, :, qcols],
                        start=True, stop=True, perf_mode=DR)
                # exp(S/16) -> bf16, one 2-bank ACT op
                nc.scalar.activation(out=e_m[:, 1024 * p:1024 * (p + 1)],
                                     in_=sp, func=AF.Exp, scale=1.0 / 16.0)

            o_pss = {}
            r_ms = {}

            def hfold(src, dst, hs, g, eng=None):
                # src [p, g, hs, 8] -> dst [p, g, hs/2, 8]  (packed-inner 2x)
                i0 = bass.AP(tensor=src.tensor, offset=src.offset,
                             ap=[src.ap[0], [hs * 8, g], [8, hs // 2], [1, 8]])
                i1 = bass.AP(tensor=src.tensor,
                             offset=src.offset + (hs // 2) * 8,
                             ap=[src.ap[0], [hs * 8, g], [8, hs // 2], [1, 8]])
                o = bass.AP(tensor=dst.tensor, offset=dst.offset,
                            ap=[dst.ap[0], [hs * 4, g], [8, hs // 2], [1, 8]])
                (eng or nc.vector).tensor_add(out=o, in0=i0, in1=i1)

            def emit_chain(qt, g):
                """softmax denominators for half g of qt (DVE fold chain)."""
                e_m = e_ms[qt]
                if g == 0:
                    o_pss[qt] = [pop.tile([128, 512], F32, tag="o",
                                          name=f"ops{qt}_{ct}")
                                 for ct in range(2)]
                if kstage == "attn_nonorm":
                    return
                HG = 16          # (pair, t) groups in a half
                e_h = e_m[:, 8192 * g:8192 * (g + 1)]
                u1 = pu_pool.tile([128, 4096], BF16, tag="u1",
                                  name=f"u1_{qt}_{g}")
                u2 = pu_pool.tile([128, 2048], BF16, tag="u2",
                                  name=f"u2_{qt}_{g}")
                d_m = pd_pool.tile([128, 128], F32, tag="d", name=f"d{qt}_{g}")
                r_m = pd_pool.tile([128, 128], BF16, tag="r", name=f"r{qt}_{g}")
                r_ms[(qt, g)] = r_m
                hfold(e_h, u1, 64, HG)                 # h 64 -> 32
                hfold(u1, u2, 32, HG)                  # 32 -> 16
                hfold(u2, u1[:, :1024], 16, HG)        # 16 -> 8
                hfold(u1[:, :1024], u2[:, :512], 8, HG)  # 8 -> 4
                hfold(u2[:, :512], u1[:, 1024:1280], 4, HG)  # 4 -> 2
                u5 = u1[:, 1024:1280]
                i0 = bass.AP(tensor=u5.tensor, offset=u5.offset,
                             ap=[u5.ap[0], [16, HG], [1, 8]])
                i1 = bass.AP(tensor=u5.tensor, offset=u5.offset + 8,
                             ap=[u5.ap[0], [16, HG], [1, 8]])
                nc.vector.tensor_add(out=d_m, in0=i0, in1=i1)
                nc.vector.reciprocal(out=r_m, in_=d_m)

            def emit_consume_pair(qt, p):
                e_m = e_ms[qt]
                o_ps = o_pss[qt]
                p0 = 8 * (p // 8)
                if kstage != "attn_nonorm":
                    r_m = r_ms[(qt, p // 8)]
                    # attn = e*r (bcast over h via middle axis; 2x_1p)
                    e4 = bass.AP(tensor=e_m.tensor,
                                 offset=e_m.offset + 1024 * p,
                                 ap=[e_m.ap[0], [512, 2], [8, 64], [1, 8]])
                    r4 = bass.AP(tensor=r_m.tensor,
                                 offset=r_m.offset + 16 * (p - p0),
                                 ap=[r_m.ap[0], [8, 2], [0, 64], [1, 8]])
                    nc.vector.tensor_mul(out=e4, in0=e4, in1=r4)
                for ct in range(2):
                    for t01 in range(2):
                        nc.tensor.matmul(
                            o_ps[ct],
                            lhsT=vT[2 * p + t01][:, 128 * ct:128 * (ct + 1)],
                            rhs=e_m[:, 1024 * p + 512 * t01:
                                    1024 * p + 512 * (t01 + 1)],
                            start=(p == 0 and t01 == 0),
                            stop=(p == NPAIR - 1 and t01 == 1))

            def emit_finish(qt):
                o_ps = o_pss.pop(qt)
                del e_ms[qt]
                r_ms.pop((qt, 0), None)
                r_ms.pop((qt, 1), None)
                qcols = slice(512 * qt, 512 * (qt + 1))
                for ct in range(2):
                    nc.scalar.activation(out=o_sb[ct][:, qcols], in_=o_ps[ct],
                                         func=AF.Identity)
                # out-proj + residual for this quarter
                for ct in range(2):
                    prj_t = psum_t(f"prj{qt}_{ct}")
                    prj = prj_t[:, :512]
                    for i in range(2):
                        nc.tensor.matmul(
                            prj,
                            lhsT=wo_bf[i][:, 128 * ct:128 * (ct + 1)],
                            rhs=o_sb[i][:, qcols],
                            start=(i == 0), stop=(i == 1))
                    ot = pout.tile([128, 512], F32, tag="ot",
                                   name=f"ot{qt}_{ct}")
                    nc.vector.scalar_tensor_tensor(
                        out=ot, in0=prj, scalar=bcol["o"][ct],
                        in1=xh[ct][:, qcols].bitcast(F32),
                        op0=ALU.add, op1=ALU.add)
                    nc.sync.dma_start(out=out_d[128 * ct:128 * (ct + 1), qcols],
                                      in_=ot)

            if kstage == "conv":
                for rt in range(N // 128):
                    emit_vt(rt)
            if kstage != "conv":
                # pair-granular software pipeline: S-MMs of qt interleave
                # with O-MMs of qt-1 so exp-throttled S never idles the PE;
                # the V^T conv fills the PE during the first produce phase
                for p in range(NPAIR):
                    emit_produce_pair(0, p)
                    emit_vt(2 * p)
                    emit_vt(2 * p + 1)
                for qt in range(1, 4):
                    emit_chain(qt - 1, 0)
                    for p in range(NPAIR):
                        emit_produce_pair(qt, p)
                        if p == 8:
                            emit_chain(qt - 1, 1)
                        if p >= 2:
                            emit_consume_pair(qt - 1, p - 2)
                    for p in range(NPAIR - 2, NPAIR):
                        emit_consume_pair(qt - 1, p)
                    emit_finish(qt - 1)
                emit_chain(3, 0)
                emit_chain(3, 1)
                for p in range(NPAIR):
                    emit_consume_pair(3, p)
                emit_finish(3)
    nc.compile()
    return nc


_NC = None


def _get_nc():
    global _NC
    if _NC is None:
        _NC = build_nc()
    return _NC


def _prep_in_maps(x, gamma, beta, q_w, q_b, k_w, k_b, v_w, v_b, o_w, o_b):
    x = np.ascontiguousarray(np.asarray(x, np.float32))
    g1 = np.zeros((C, GROUPS), np.float32)
    g1[np.arange(C), np.arange(C) // (C // GROUPS)] = 1.0
    shared = {
        "gamma_c": np.asarray(gamma, np.float32).reshape(C, 1).copy(),
        "beta_c": np.asarray(beta, np.float32).reshape(C, 1).copy(),
        "G1": g1,
        "G2": np.ascontiguousarray(g1.T),
        "ones_row": np.ones((1, 512), np.float32),
    }
    for t, wm, bv in (("q", q_w, q_b), ("k", k_w, k_b),
                      ("v", v_w, v_b), ("o", o_w, o_b)):
        shared[f"w{t}T"] = np.ascontiguousarray(np.asarray(wm, np.float32).T)
        if t == "v":
            shared["bv_row"] = np.asarray(bv, np.float32).reshape(1, C).copy()
        else:
            shared[f"b{t}_col"] = np.asarray(bv, np.float32).reshape(C, 1).copy()
    in_maps = []
    import ml_dtypes
    for core in range(8):
        b, half = core // 2, core % 2
        xb = np.ascontiguousarray(x[b].reshape(C, N)).astype(ml_dtypes.bfloat16)
        # queries h-major inside each 512 block: q = qt*512 + h*8 + ww
        xh = x[b][:, :, half * WH:(half + 1) * WH]           # [C, 64h, 32w']
        xh = np.ascontiguousarray(
            xh.reshape(C, H, 4, 8).transpose(0, 2, 1, 3)
        ).reshape(C, NH)
        in_maps.append(dict(shared, xf=np.ascontiguousarray(xb), xh=xh))
    return in_maps


def run(trace=False, **inputs):
    in_maps = _prep_in_maps(**inputs)
    nc = _get_nc()
    res = run_bass_kernel_spmd(nc, in_maps, core_ids=list(range(8)), trace=trace)
    x = np.asarray(inputs["x"], np.float32)
    out = np.empty((B, C, H, W), np.float32)
    for core in range(8):
        b, half = core // 2, core % 2
        od = res.results[core]["out"]                        # [C, 2048]
        oh = od.reshape(C, 4, H, 8).transpose(0, 2, 1, 3).reshape(C, H, WH)
        out[b][:, :, half * WH:(half + 1) * WH] = oh
    return out, res


def kernel(**inputs):
    out, _ = run(trace=False, **inputs)
    return out


# revision 30
# speedup vs baseline: 1.2830x; 1.2830x over previous
"""AttnBlock (GroupNorm + 1x1-conv QKV + spatial attention w/ softmax over
query-h + out-proj + residual) for Trainium2, 8 NeuronCores.

Sharding: core = 2*b + w_half  (4 samples x 2 halves of the w axis).
The softmax normalizes over the h index of the *query*, so for a fixed w
column the 64 h-values form one softmax group; splitting by w keeps every
group on one core.

v2 design (vs the fp32r baseline):
  - Query packing is h-major inside each 512-query block: q = qt*512 + h*8
    + ww (ww = w' - 8*qt). The softmax-normalize multiply then has its
    broadcast on a *middle* axis and packed bf16 innermost, which unlocks
    the DVE 2x_1p fast mode.
  - S = K^T Q runs as ONE fp8e4 DoubleRow matmul per (key-block, q-block):
    both operands are [ki, 2, *] with channel c = t*128 + ki. 2x PE rate.
  - exp reads a 2-bank [128, 1024] PSUM span in one ACT op, writes bf16.
  - d (softmax denominators, per (key, w')) via GpSimd half-fold (bf16
    tensor_add) + DVE strided segmented reduce; reciprocal on DVE (bf16).
  - attn = e * r broadcast-multiply on DVE at 2x (a few pairs on GpSimd).
  - O = V^T attn accumulates in bf16 (V^T tiles cast to bf16 at conv time).
  - GroupNorm is folded into the conv weights on device (as baseline).
"""

import os

import numpy as np

import concourse.bass as bass
import concourse.bacc as bacc
import concourse.mybir as mybir
import concourse.tile as tile
from concourse.bass_utils import run_bass_kernel_spmd

B, C, H, W = 4, 256, 64, 64
N = H * W            # 4096 keys
NH = N // 2          # 2048 queries per w-half
WH = W // 2          # 32 local w' values
GROUPS = 32
EPS = 1e-5
F32 = mybir.dt.float32
F32R = mybir.dt.float32r
BF16 = mybir.dt.bfloat16
FP8 = mybir.dt.float8e4
AF = mybir.ActivationFunctionType
ALU = mybir.AluOpType
AX = mybir.AxisListType
DR = mybir.MatmulPerfMode.DoubleRow


def _r(ap):
    return ap.bitcast(F32R)


def _bcast_mid(ap, n):
    """[p, ..., m] AP -> [p, ..., 0 x n, m]: broadcast over a new middle
    axis, keeping the packed innermost dim (preserves DVE 2x_1p)."""
    return bass.AP(tensor=ap.tensor, offset=ap.offset,
                   ap=[*ap.ap[:-1], [0, n], ap.ap[-1]])


def build_nc():
    nc = bacc.Bacc("TRN2", target_bir_lowering=False, debug=False)

    xf_d = nc.dram_tensor("xf", [C, N], BF16, kind="ExternalInput")
    xh_d = nc.dram_tensor("xh", [C, NH], F32, kind="ExternalInput")
    wT_d = {t: nc.dram_tensor(f"w{t}T", [C, C], F32, kind="ExternalInput")
            for t in "qkvo"}
    brow_d = {"v": nc.dram_tensor("bv_row", [1, C], F32, kind="ExternalInput")}
    bcol_d = {t: nc.dram_tensor(f"b{t}_col", [C, 1], F32, kind="ExternalInput")
              for t in "qko"}
    gamma_d = nc.dram_tensor("gamma_c", [C, 1], F32, kind="ExternalInput")
    beta_d = nc.dram_tensor("beta_c", [C, 1], F32, kind="ExternalInput")
    g1_d = nc.dram_tensor("G1", [C, GROUPS], F32, kind="ExternalInput")
    g2_d = nc.dram_tensor("G2", [GROUPS, C], F32, kind="ExternalInput")
    ones_d = nc.dram_tensor("ones_row", [1, 512], F32, kind="ExternalInput")
    out_d = nc.dram_tensor("out", [C, NH], F32, kind="ExternalOutput")

    with tile.TileContext(nc) as tc:
        with (
            nc.allow_low_precision(reason="bf16 softmax pipeline, 2e-2 gate"),
            tc.tile_pool(name="persist", bufs=1) as pp,
            tc.tile_pool(name="mm", bufs=3, space="PSUM") as pmm,
            tc.tile_pool(name="opsum", bufs=2, space="PSUM") as pop,
            tc.tile_pool(name="epool", bufs=2) as pe_pool,
            tc.tile_pool(name="upool", bufs=1) as pu_pool,
            tc.tile_pool(name="dpool", bufs=2) as pd_pool,
            tc.tile_pool(name="gnpool", bufs=2) as pgn_pool,
            tc.tile_pool(name="outpool", bufs=3) as pout,
        ):
            def ptile(shape, tag, dtype=F32):
                return pp.tile(shape, dtype, tag=tag, name=tag)

            def psum_t(tag_name):
                # [128, 1024] fp32 = 2 PSUM banks
                return pmm.tile([128, 1024], F32, tag="mm", name=tag_name)

            # ---------------- loads ----------------
            xf = []
            xh = []
            wT = {t: [] for t in "qkvo"}
            gam, bet, g1 = [], [], []
            for i in range(2):
                t = ptile([128, N], f"xf{i}", BF16)
                for ch in range(4):
                    nc.sync.dma_start(
                        out=t[:, 1024 * ch:1024 * (ch + 1)],
                        in_=xf_d[128 * i:128 * (i + 1),
                                 1024 * ch:1024 * (ch + 1)])
                xf.append(t)
            for i in range(2):
                t = ptile([128, NH], f"xh{i}", F32R)
                for ch in range(2):
                    nc.sync.dma_start(
                        out=t[:, 1024 * ch:1024 * (ch + 1)],
                        in_=xh_d[128 * i:128 * (i + 1),
                                 1024 * ch:1024 * (ch + 1)].bitcast(F32R))
                xh.append(t)
            for i in range(2):
                for w in "qkvo":
                    t = ptile([128, C], f"w{w}T{i}", F32R)
                    nc.sync.dma_start(out=t, in_=wT_d[w][128 * i:128 * (i + 1), :].bitcast(F32R))
                    wT[w].append(t)
                t = ptile([128, 1], f"gam{i}")
                nc.sync.dma_start(out=t, in_=gamma_d[128 * i:128 * (i + 1), :])
                gam.append(t)
                t = ptile([128, 1], f"bet{i}")
                nc.sync.dma_start(out=t, in_=beta_d[128 * i:128 * (i + 1), :])
                bet.append(t)
                t = ptile([128, GROUPS], f"g1_{i}")
                nc.sync.dma_start(out=t, in_=g1_d[128 * i:128 * (i + 1), :])
                g1.append(t)
            g2 = ptile([GROUPS, C], "g2")
            nc.sync.dma_start(out=g2, in_=g2_d[:, :])
            ones = ptile([1, 512], "ones", F32R)
            nc.sync.dma_start(out=ones, in_=ones_d[:, :].bitcast(F32R))
            brow = {}
            for w in "v":
                brow[w] = ptile([1, C], f"b{w}row", F32R)
                nc.sync.dma_start(out=brow[w], in_=brow_d[w][:, :].bitcast(F32R))
            bcol = {}
            for w in "qko":
                bcol[w] = []
                for i in range(2):
                    t = ptile([128, 1], f"b{w}col{i}")
                    nc.sync.dma_start(out=t, in_=bcol_d[w][128 * i:128 * (i + 1), :])
                    bcol[w].append(t)

            # ---------------- GroupNorm stats -> per-channel scale/shift ----
            NSUB = N // 512
            mstat = []
            for i in range(2):
                stats = pgn_pool.tile([128, NSUB, 6], F32, tag="gnstats",
                                     name=f"gnstats{i}")
                for s in range(NSUB):
                    nc.vector.bn_stats(out=stats[:, s, :],
                                       in_=xf[i][:, 512 * s:512 * (s + 1)])
                mv = pgn_pool.tile([128, 2], F32, tag="gnmv", name=f"gnmv{i}")
                nc.vector.bn_aggr(out=mv, in_=stats)
                ms = ptile([128, 2], f"mstat{i}")
                # ms = [mean_c, E[x^2]_c]
                nc.vector.tensor_mul(out=ms[:, 1:2], in0=mv[:, 0:1], in1=mv[:, 0:1])
                nc.vector.tensor_add(out=ms[:, 1:2], in0=ms[:, 1:2], in1=mv[:, 1:2])
                nc.vector.tensor_copy(out=ms[:, 0:1], in_=mv[:, 0:1])
                mstat.append(ms)

            pg_t = psum_t("pg")
            pg = pg_t[:GROUPS, :2]
            for i in range(2):
                nc.tensor.matmul(pg, lhsT=g1[i], rhs=mstat[i],
                                 start=(i == 0), stop=(i == 1))
            gstat = ptile([GROUPS, 2], "gstat")
            nc.vector.tensor_scalar_mul(out=gstat, in0=pg, scalar1=1.0 / 8.0)
            var32 = ptile([GROUPS, 1], "var32")
            nc.vector.tensor_mul(out=var32, in0=gstat[:, 0:1], in1=gstat[:, 0:1])
            nc.vector.tensor_sub(out=var32, in0=gstat[:, 1:2], in1=var32)
            std32 = ptile([GROUPS, 1], "std32")
            eps_t = ptile([GROUPS, 1], "eps_t")
            nc.vector.memset(eps_t, EPS)
            nc.scalar.activation(out=std32, in_=var32, func=AF.Sqrt, bias=eps_t)
            rstd = ptile([GROUPS, 1], "rstd")
            nc.vector.reciprocal(out=rstd, in_=std32)

            grstat = ptile([GROUPS, 2], "grstat")
            nc.vector.tensor_copy(out=grstat[:, 0:1], in_=gstat[:, 0:1])
            nc.vector.tensor_copy(out=grstat[:, 1:2], in_=rstd)

            sc, sh = [], []
            for i in range(2):
                pc_t = psum_t(f"pc{i}")
                pc = pc_t[:128, :2]
                nc.tensor.matmul(pc, lhsT=g2[:, 128 * i:128 * (i + 1)],
                                 rhs=grstat, start=True, stop=True)
                s = ptile([128, 1], f"sc{i}")
                nc.vector.tensor_mul(out=s, in0=pc[:, 1:2], in1=gam[i])
                sc.append(s)
                h = ptile([128, 1], f"sh{i}", F32R)
                nc.vector.tensor_mul(out=h, in0=pc[:, 0:1], in1=s)
                nc.vector.tensor_sub(out=h, in0=bet[i], in1=h)
                sh.append(h)

            # effective v bias as a row (per-free-column bias for V^T)
            beffr = {}
            for w in "v":
                rp_t = psum_t(f"br{w}")
                rp = rp_t[:1, :C]
                for i in range(2):
                    nc.tensor.matmul(rp, lhsT=sh[i], rhs=wT[w][i],
                                     start=(i == 0), stop=(i == 1))
                bt = ptile([1, C], f"beff{w}", F32R)
                nc.vector.tensor_add(out=bt, in0=rp, in1=brow[w])
                beffr[w] = bt
            # effective q,k biases as columns (per-partition bias for ACT fuse)
            beffc = {}
            for w in "qk":
                beffc[w] = []
                for j in range(2):
                    bp_t = psum_t(f"bc{w}{j}")
                    bp = bp_t[:128, :1]
                    for i in range(2):
                        nc.tensor.matmul(bp,
                                         lhsT=wT[w][i][:, 128 * j:128 * (j + 1)].bitcast(F32),
                                         rhs=sh[i].bitcast(F32),
                                         start=(i == 0), stop=(i == 1))
                    t = ptile([128, 1], f"beffc{w}{j}")
                    nc.vector.tensor_add(out=t, in0=bp, in1=bcol[w][j])
                    beffc[w].append(t)

            # fold GN scale into conv weights: q in place (f32r),
            # k/v as scaled bf16 copies (convs run bf16 against bf16 xf)
            for i in range(2):
                nc.vector.tensor_scalar_mul(out=wT["q"][i], in0=wT["q"][i],
                                            scalar1=sc[i])
            w_bf = {}
            for w in "kv":
                w_bf[w] = []
                for i in range(2):
                    t = ptile([128, C], f"w{w}bf{i}", BF16)
                    nc.scalar.activation(out=t, in_=wT[w][i].bitcast(F32),
                                         func=AF.Identity, scale=sc[i])
                    w_bf[w].append(t)
            ones_bf = ptile([1, 128], "ones_bf", BF16)
            nc.vector.memset(ones_bf, 1.0)
            beffr_bf = ptile([1, C], "beffr_bf", BF16)
            nc.vector.tensor_copy(out=beffr_bf, in_=beffr["v"].bitcast(F32))

            # ---------------- convs: K, Q (fp8 DoubleRow layout), V^T (bf16)
            # k8/q8 layout [ki, t, col]: channel c = t*128 + ki
            k8 = ptile([128, 2, N], "k8", FP8)
            q8 = ptile([128, 2, NH], "q8", FP8)
            for s2 in range(N // 1024):
                for j in range(2):
                    kp = psum_t(f"kp{j}_{s2}")
                    for half in range(2):
                        cols = slice(1024 * s2 + 512 * half,
                                     1024 * s2 + 512 * (half + 1))
                        for i in range(2):
                            nc.tensor.matmul(
                                kp[:, 512 * half:512 * (half + 1)],
                                lhsT=w_bf["k"][i][:, 128 * j:128 * (j + 1)],
                                rhs=xf[i][:, cols],
                                start=(i == 0), stop=(i == 1))
                    nc.scalar.activation(
                        out=k8[:, j, 1024 * s2:1024 * (s2 + 1)],
                        in_=kp, func=AF.Identity, bias=beffc["k"][j])
            for s2 in range(NH // 1024):
                for j in range(2):
                    qp = psum_t(f"qp{j}_{s2}")
                    for half in range(2):
                        cols = slice(1024 * s2 + 512 * half,
                                     1024 * s2 + 512 * (half + 1))
                        for i in range(2):
                            nc.tensor.matmul(
                                qp[:, 512 * half:512 * (half + 1)],
                                lhsT=_r(wT["q"][i][:, 128 * j:128 * (j + 1)]),
                                rhs=_r(xh[i][:, cols]),
                                start=(i == 0), stop=(i == 1))
                    nc.scalar.activation(
                        out=q8[:, j, 1024 * s2:1024 * (s2 + 1)],
                        in_=qp, func=AF.Identity, bias=beffc["q"][j])

            vT = [ptile([128, C], f"vT{rt}", BF16) for rt in range(N // 128)]

            def emit_vt(rt):
                # own PSUM tag: vp must not steal sp-pool slots during
                # produce(0); the o_ps banks are still free at that point
                vp_t = pop.tile([128, 512], F32, tag="o", name=f"vp{rt}")
                vp = vp_t[:, :C]
                for i in range(2):
                    nc.tensor.matmul(vp,
                                     lhsT=xf[i][:, 128 * rt:128 * (rt + 1)],
                                     rhs=w_bf["v"][i],
                                     start=(i == 0), stop=False)
                nc.tensor.matmul(vp, lhsT=ones_bf, rhs=beffr_bf,
                                 start=False, stop=True)
                t = vT[rt]
                if rt % 4 == 0:
                    nc.scalar.activation(out=t, in_=vp, func=AF.Identity)
                else:
                    nc.vector.tensor_copy(out=t, in_=vp)

            # bf16 copy of out-proj weights (O path runs bf16)
            wo_bf = []
            for i in range(2):
                t = ptile([128, C], f"wo_bf{i}", BF16)
                nc.vector.tensor_copy(out=t, in_=wT["o"][i].bitcast(F32))
                wo_bf.append(t)

            o_sb = [ptile([128, NH], "o_sb0", BF16), ptile([128, NH], "o_sb1", BF16)]

            kstage = os.environ.get("KSTAGE", "full")
            if kstage == "conv":
                for ct in range(2):
                    dbg = pout.tile([128, NH], F32, tag="dbg", name=f"dbg{ct}",
                                    bufs=2)
                    nc.vector.tensor_copy(out=dbg, in_=k8[:, ct, :NH])
                    nc.sync.dma_start(out=out_d[128 * ct:128 * (ct + 1), :],
                                      in_=dbg)

            # ---------------- attention (qt-phased, SW-pipelined) ----------
            # e_mega layout: idx = pair*1024 + t*512 + h*8 + ww
            # u_mega layout: idx = pair*512 + t*256 + ww*32 + h'
            # d/r layout:    idx = pair*16 + t*8 + ww
            NPAIR = N // 256     # 16 pairs of 128-key blocks
            NGP = 12             # pairs folded on GpSimd; rest on DVE
            NCH = 4              # pairs per reduce chunk
            e_ms = {}

            def emit_produce(qt):
                e_m = pe_pool.tile([128, NPAIR * 1024], BF16, tag="emega",
                                   name=f"em{qt}")
                e_ms[qt] = e_m
                for p in range(NPAIR):
                    emit_produce_pair(qt, p)

            def emit_produce_pair(qt, p):
                if p == 0 and qt not in e_ms:
                    e_ms[qt] = pe_pool.tile([128, NPAIR * 1024], BF16,
                                            tag="emega", name=f"em{qt}")
                e_m = e_ms[qt]
                qcols = slice(512 * qt, 512 * (qt + 1))
                sp = psum_t(f"sp{qt}_{p}")
                for t01 in range(2):
                    kb = 256 * p + 128 * t01
                    nc.tensor.matmul(
                        sp[:, 512 * t01:512 * (t01 + 1)],
                        lhsT=k8[:, :, kb:kb + 128],
                        rhs=q8[:, :, qcols],
                        start=True, stop=True, perf_mode=DR)
                # exp(S/16) -> bf16, one 2-bank ACT op
                nc.scalar.activation(out=e_m[:, 1024 * p:1024 * (p + 1)],
                                     in_=sp, func=AF.Exp, scale=1.0 / 16.0)

            o_pss = {}
            r_ms = {}

            def hfold(src, dst, hs, g, eng=None):
                # src [p, g, hs, 8] -> dst [p, g, hs/2, 8]  (packed-inner 2x)
                i0 = bass.AP(tensor=src.tensor, offset=src.offset,
                             ap=[src.ap[0], [hs * 8, g], [8, hs // 2], [1, 8]])
                i1 = bass.AP(tensor=src.tensor,
                             offset=src.offset + (hs // 2) * 8,
                             ap=[src.ap[0], [hs * 8, g], [8, hs // 2], [1, 8]])
                o = bass.AP(tensor=dst.tensor, offset=dst.offset,
                            ap=[dst.ap[0], [hs * 4, g], [8, hs // 2], [1, 8]])
                (eng or nc.vector).tensor_add(out=o, in0=i0, in1=i1)

            def emit_chain(qt, g):
                """softmax denominators for half g of qt (DVE fold chain)."""
                e_m = e_ms[qt]
                if g == 0:
                    o_pss[qt] = [pop.tile([128, 512], F32, tag="o",
                                          name=f"ops{qt}_{ct}")
                                 for ct in range(2)]
                if kstage == "attn_nonorm":
                    return
                HG = 16          # (pair, t) groups in a half
                e_h = e_m[:, 8192 * g:8192 * (g + 1)]
                u1 = pu_pool.tile([128, 4096], BF16, tag="u1",
                                  name=f"u1_{qt}_{g}")
                u2 = pu_pool.tile([128, 2048], BF16, tag="u2",
                                  name=f"u2_{qt}_{g}")
                d_m = pd_pool.tile([128, 128], F32, tag="d", name=f"d{qt}_{g}")
                r_m = pd_pool.tile([128, 128], BF16, tag="r", name=f"r{qt}_{g}")
                r_ms[(qt, g)] = r_m
                hfold(e_h, u1, 64, HG)                 # h 64 -> 32
                hfold(u1, u2, 32, HG)                  # 32 -> 16
                hfold(u2, u1[:, :1024], 16, HG)        # 16 -> 8
                hfold(u1[:, :1024], u2[:, :512], 8, HG)  # 8 -> 4
                hfold(u2[:, :512], u1[:, 1024:1280], 4, HG)  # 4 -> 2
                u5 = u1[:, 1024:1280]
                i0 = bass.AP(tensor=u5.tensor, offset=u5.offset,
                             ap=[u5.ap[0], [16, HG], [1, 8]])
                i1 = bass.AP(tensor=u5.tensor, offset=u5.offset + 8,
                             ap=[u5.ap[0], [16, HG], [1, 8]])
                nc.vector.tensor_add(out=d_m, in0=i0, in1=i1)
                nc.vector.reciprocal(out=r_m, in_=d_m)

            def emit_consume_pair(qt, p):
                e_m = e_ms[qt]
                o_ps = o_pss[qt]
                p0 = 8 * (p // 8)
                if kstage != "attn_nonorm":
                    r_m = r_ms[(qt, p // 8)]
                    # attn = e*r (bcast over h via middle axis; 2x_1p)
                    e4 = bass.AP(tensor=e_m.tensor,
                                 offset=e_m.offset + 1024 * p,
                                 ap=[e_m.ap[0], [512, 2], [8, 64], [1, 8]])
                    r4 = bass.AP(tensor=r_m.tensor,
                                 offset=r_m.offset + 16 * (p - p0),
                                 ap=[r_m.ap[0], [8, 2], [0, 64], [1, 8]])
                    nc.vector.tensor_mul(out=e4, in0=e4, in1=r4)
                for ct in range(2):
                    for t01 in range(2):
                        nc.tensor.matmul(
                            o_ps[ct],
                            lhsT=vT[2 * p + t01][:, 128 * ct:128 * (ct + 1)],
                            rhs=e_m[:, 1024 * p + 512 * t01:
                                    1024 * p + 512 * (t01 + 1)],
                            start=(p == 0 and t01 == 0),
                            stop=(p == NPAIR - 1 and t01 == 1))

            def emit_finish(qt):
                o_ps = o_pss.pop(qt)
                del e_ms[qt]
                r_ms.pop((qt, 0), None)
                r_ms.pop((qt, 1), None)
                qcols = slice(512 * qt, 512 * (qt + 1))
                for ct in range(2):
                    nc.scalar.activation(out=o_sb[ct][:, qcols], in_=o_ps[ct],
                                         func=AF.Identity)
                # out-proj + residual for this quarter
                for ct in range(2):
                    prj_t = psum_t(f"prj{qt}_{ct}")
                    prj = prj_t[:, :512]
                    for i in range(2):
                        nc.tensor.matmul(
                            prj,
                            lhsT=wo_bf[i][:, 128 * ct:128 * (ct + 1)],
                            rhs=o_sb[i][:, qcols],
                            start=(i == 0), stop=(i == 1))
                    ot = pout.tile([128, 512], F32, tag="ot",
                                   name=f"ot{qt}_{ct}")
                    nc.vector.scalar_tensor_tensor(
                        out=ot, in0=prj, scalar=bcol["o"][ct],
                        in1=xh[ct][:, qcols].bitcast(F32),
                        op0=ALU.add, op1=ALU.add)
                    nc.sync.dma_start(out=out_d[128 * ct:128 * (ct + 1), qcols],
                                      in_=ot)

            if kstage == "conv":
                for rt in range(N // 128):
                    emit_vt(rt)
            if kstage != "conv":
                # pair-granular software pipeline: S-MMs of qt interleave
                # with O-MMs of qt-1 so exp-throttled S never idles the PE;
                # the V^T conv fills the PE during the first produce phase
                for p in range(NPAIR):
                    emit_produce_pair(0, p)
                    emit_vt(2 * p)
                    emit_vt(2 * p + 1)
                for qt in range(1, 4):
                    emit_chain(qt - 1, 0)
                    for p in range(NPAIR):
                        emit_produce_pair(qt, p)
                        if p == 8:
                            emit_chain(qt - 1, 1)
                        if p >= 2:
                            emit_consume_pair(qt - 1, p - 2)
                    for p in range(NPAIR - 2, NPAIR):
                        emit_consume_pair(qt - 1, p)
                    emit_finish(qt - 1)
                emit_chain(3, 0)
                emit_chain(3, 1)
                for p in range(NPAIR):
                    emit_consume_pair(3, p)
                emit_finish(3)
    nc.compile()
    return nc


_NC = None


def _get_nc():
    global _NC
    if _NC is None:
        _NC = build_nc()
    return _NC


def _prep_in_maps(x, gamma, beta, q_w, q_b, k_w, k_b, v_w, v_b, o_w, o_b):
    x = np.ascontiguousarray(np.asarray(x, np.float32))
    g1 = np.zeros((C, GROUPS), np.float32)
    g1[np.arange(C), np.arange(C) // (C // GROUPS)] = 1.0
    shared = {
        "gamma_c": np.asarray(gamma, np.float32).reshape(C, 1).copy(),
        "beta_c": np.asarray(beta, np.float32).reshape(C, 1).copy(),
        "G1": g1,
        "G2": np.ascontiguousarray(g1.T),
        "ones_row": np.ones((1, 512), np.float32),
    }
    for t, wm, bv in (("q", q_w, q_b), ("k", k_w, k_b),
                      ("v", v_w, v_b), ("o", o_w, o_b)):
        shared[f"w{t}T"] = np.ascontiguousarray(np.asarray(wm, np.float32).T)
        if t == "v":
            shared["bv_row"] = np.asarray(bv, np.float32).reshape(1, C).copy()
        else:
            shared[f"b{t}_col"] = np.asarray(bv, np.float32).reshape(C, 1).copy()
    in_maps = []
    import ml_dtypes
    for core in range(8):
        b, half = core // 2, core % 2
        xb = np.ascontiguousarray(x[b].reshape(C, N)).astype(ml_dtypes.bfloat16)
        # queries h-major inside each 512 block: q = qt*512 + h*8 + ww
        xh = x[b][:, :, half * WH:(half + 1) * WH]           # [C, 64h, 32w']
        xh = np.ascontiguousarray(
            xh.reshape(C, H, 4, 8).transpose(0, 2, 1, 3)
        ).reshape(C, NH)
        in_maps.append(dict(shared, xf=np.ascontiguousarray(xb), xh=xh))
    return in_maps


def run(trace=False, **inputs):
    in_maps = _prep_in_maps(**inputs)
    nc = _get_nc()
    res = run_bass_kernel_spmd(nc, in_maps, core_ids=list(range(8)), trace=trace)
    x = np.asarray(inputs["x"], np.float32)
    out = np.empty((B, C, H, W), np.float32)
    for core in range(8):
        b, half = core // 2, core % 2
        od = res.results[core]["out"]                        # [C, 2048]
        oh = od.reshape(C, 4, H, 8).transpose(0, 2, 1, 3).reshape(C, H, WH)
        out[b][:, :, half * WH:(half + 1) * WH] = oh
    return out, res


def kernel(**inputs):
    out, _ = run(trace=False, **inputs)
    return out


# revision 32
# speedup vs baseline: 1.3075x; 1.0191x over previous
"""AttnBlock (GroupNorm + 1x1-conv QKV + spatial attention w/ softmax over
query-h + out-proj + residual) for Trainium2, 8 NeuronCores.

Sharding: core = 2*b + w_half  (4 samples x 2 halves of the w axis).
The softmax normalizes over the h index of the *query*, so for a fixed w
column the 64 h-values form one softmax group; splitting by w keeps every
group on one core.

v2 design (vs the fp32r baseline):
  - Query packing is h-major inside each 512-query block: q = qt*512 + h*8
    + ww (ww = w' - 8*qt). The softmax-normalize multiply then has its
    broadcast on a *middle* axis and packed bf16 innermost, which unlocks
    the DVE 2x_1p fast mode.
  - S = K^T Q runs as ONE fp8e4 DoubleRow matmul per (key-block, q-block):
    both operands are [ki, 2, *] with channel c = t*128 + ki. 2x PE rate.
  - exp reads a 2-bank [128, 1024] PSUM span in one ACT op, writes bf16.
  - d (softmax denominators, per (key, w')) via GpSimd half-fold (bf16
    tensor_add) + DVE strided segmented reduce; reciprocal on DVE (bf16).
  - attn = e * r broadcast-multiply on DVE at 2x (a few pairs on GpSimd).
  - O = V^T attn accumulates in bf16 (V^T tiles cast to bf16 at conv time).
  - GroupNorm is folded into the conv weights on device (as baseline).
"""

import os

import numpy as np

import concourse.bass as bass
import concourse.bacc as bacc
import concourse.mybir as mybir
import concourse.tile as tile
from concourse.bass_utils import run_bass_kernel_spmd

B, C, H, W = 4, 256, 64, 64
N = H * W            # 4096 keys
NH = N // 2          # 2048 queries per w-half
WH = W // 2          # 32 local w' values
GROUPS = 32
EPS = 1e-5
F32 = mybir.dt.float32
F32R = mybir.dt.float32r
BF16 = mybir.dt.bfloat16
FP8 = mybir.dt.float8e4
AF = mybir.ActivationFunctionType
ALU = mybir.AluOpType
AX = mybir.AxisListType
DR = mybir.MatmulPerfMode.DoubleRow


def _r(ap):
    return ap.bitcast(F32R)


def _bcast_mid(ap, n):
    """[p, ..., m] AP -> [p, ..., 0 x n, m]: broadcast over a new middle
    axis, keeping the packed innermost dim (preserves DVE 2x_1p)."""
    return bass.AP(tensor=ap.tensor, offset=ap.offset,
                   ap=[*ap.ap[:-1], [0, n], ap.ap[-1]])


def build_nc():
    nc = bacc.Bacc("TRN2", target_bir_lowering=False, debug=False)

    xf_d = nc.dram_tensor("xf", [C, N], BF16, kind="ExternalInput")
    xh_d = nc.dram_tensor("xh", [C, NH], F32, kind="ExternalInput")
    # packed params: wall [C,4C] = q|k|v|o transposed weights;
    # pcols [C,40] = gamma|beta|bq|bk|bo|G1(32)|pad; smalls [34,512] =
    # ones row | bv_row(256)+pad | G2 rows (cols 0:256)
    wall_d = nc.dram_tensor("wall", [C, 4 * C], F32, kind="ExternalInput")
    pcols_d = nc.dram_tensor("pcols", [C, 40], F32, kind="ExternalInput")
    smalls_d = nc.dram_tensor("smalls", [GROUPS, C], F32, kind="ExternalInput")
    bv_d = nc.dram_tensor("bv_row", [1, C], F32, kind="ExternalInput")
    out_d = nc.dram_tensor("out", [C, NH], F32, kind="ExternalOutput")

    with tile.TileContext(nc) as tc:
        with (
            nc.allow_low_precision(reason="bf16 softmax pipeline, 2e-2 gate"),
            tc.tile_pool(name="persist", bufs=1) as pp,
            tc.tile_pool(name="mm", bufs=3, space="PSUM") as pmm,
            tc.tile_pool(name="opsum", bufs=2, space="PSUM") as pop,
            tc.tile_pool(name="epool", bufs=2) as pe_pool,
            tc.tile_pool(name="upool", bufs=1) as pu_pool,
            tc.tile_pool(name="dpool", bufs=2) as pd_pool,
            tc.tile_pool(name="gnpool", bufs=2) as pgn_pool,
            tc.tile_pool(name="outpool", bufs=3) as pout,
        ):
            def ptile(shape, tag, dtype=F32):
                return pp.tile(shape, dtype, tag=tag, name=tag)

            def psum_t(tag_name):
                # [128, 1024] fp32 = 2 PSUM banks
                return pmm.tile([128, 1024], F32, tag="mm", name=tag_name)

            # ---------------- loads ----------------
            xf = []
            xh = []
            wT = {t: [] for t in "qkvo"}
            gam, bet, g1 = [], [], []
            for i in range(2):
                t = ptile([128, N], f"xf{i}", BF16)
                for ch in range(4):
                    nc.sync.dma_start(
                        out=t[:, 1024 * ch:1024 * (ch + 1)],
                        in_=xf_d[128 * i:128 * (i + 1),
                                 1024 * ch:1024 * (ch + 1)])
                xf.append(t)
            for i in range(2):
                t = ptile([128, NH], f"xh{i}", F32R)
                for ch in range(2):
                    nc.sync.dma_start(
                        out=t[:, 1024 * ch:1024 * (ch + 1)],
                        in_=xh_d[128 * i:128 * (i + 1),
                                 1024 * ch:1024 * (ch + 1)].bitcast(F32R))
                xh.append(t)
            wall = []
            pcols = []
            for i in range(2):
                t = ptile([128, 4 * C], f"wall{i}", F32R)
                nc.sync.dma_start(out=t, in_=wall_d[128 * i:128 * (i + 1), :].bitcast(F32R))
                wall.append(t)
                t = ptile([128, 40], f"pcols{i}")
                nc.sync.dma_start(out=t, in_=pcols_d[128 * i:128 * (i + 1), :])
                pcols.append(t)
            smalls = ptile([GROUPS, C], "smalls")
            nc.sync.dma_start(out=smalls, in_=smalls_d[:, :])
            bv = ptile([1, C], "bv")
            nc.sync.dma_start(out=bv, in_=bv_d[:, :])
            for i in range(2):
                for wi, w in enumerate("qkvo"):
                    wT[w].append(wall[i][:, C * wi:C * (wi + 1)])
                gam.append(pcols[i][:, 0:1])
                bet.append(pcols[i][:, 1:2])
                g1.append(pcols[i][:, 5:37])
            g2 = smalls
            brow = {"v": bv.bitcast(F32R)}
            bcol = {w: [pcols[i][:, 2 + wi:3 + wi] for i in range(2)]
                    for wi, w in enumerate("qko")}

            # ---------------- GroupNorm stats -> per-channel scale/shift ----
            NSUB = N // 512
            mstat = []
            for i in range(2):
                stats = pgn_pool.tile([128, NSUB, 6], F32, tag="gnstats",
                                     name=f"gnstats{i}")
                for s in range(NSUB):
                    nc.vector.bn_stats(out=stats[:, s, :],
                                       in_=xf[i][:, 512 * s:512 * (s + 1)])
                mv = pgn_pool.tile([128, 2], F32, tag="gnmv", name=f"gnmv{i}")
                nc.vector.bn_aggr(out=mv, in_=stats)
                ms = ptile([128, 2], f"mstat{i}")
                # ms = [mean_c, E[x^2]_c]
                nc.vector.tensor_mul(out=ms[:, 1:2], in0=mv[:, 0:1], in1=mv[:, 0:1])
                nc.vector.tensor_add(out=ms[:, 1:2], in0=ms[:, 1:2], in1=mv[:, 1:2])
                nc.vector.tensor_copy(out=ms[:, 0:1], in_=mv[:, 0:1])
                mstat.append(ms)

            pg_t = psum_t("pg")
            pg = pg_t[:GROUPS, :2]
            for i in range(2):
                nc.tensor.matmul(pg, lhsT=g1[i], rhs=mstat[i],
                                 start=(i == 0), stop=(i == 1))
            gstat = ptile([GROUPS, 2], "gstat")
            nc.vector.tensor_scalar_mul(out=gstat, in0=pg, scalar1=1.0 / 8.0)
            var32 = ptile([GROUPS, 1], "var32")
            nc.vector.tensor_mul(out=var32, in0=gstat[:, 0:1], in1=gstat[:, 0:1])
            nc.vector.tensor_sub(out=var32, in0=gstat[:, 1:2], in1=var32)
            std32 = ptile([GROUPS, 1], "std32")
            eps_t = ptile([GROUPS, 1], "eps_t")
            nc.vector.memset(eps_t, EPS)
            nc.scalar.activation(out=std32, in_=var32, func=AF.Sqrt, bias=eps_t)
            rstd = ptile([GROUPS, 1], "rstd")
            nc.vector.reciprocal(out=rstd, in_=std32)

            grstat = ptile([GROUPS, 2], "grstat")
            nc.vector.tensor_copy(out=grstat[:, 0:1], in_=gstat[:, 0:1])
            nc.vector.tensor_copy(out=grstat[:, 1:2], in_=rstd)

            sc, sh = [], []
            for i in range(2):
                pc_t = psum_t(f"pc{i}")
                pc = pc_t[:128, :2]
                nc.tensor.matmul(pc, lhsT=g2[:, 128 * i:128 * (i + 1)],
                                 rhs=grstat, start=True, stop=True)
                s = ptile([128, 1], f"sc{i}")
                nc.vector.tensor_mul(out=s, in0=pc[:, 1:2], in1=gam[i])
                sc.append(s)
                h = ptile([128, 1], f"sh{i}", F32R)
                nc.vector.tensor_mul(out=h, in0=pc[:, 0:1], in1=s)
                nc.vector.tensor_sub(out=h, in0=bet[i], in1=h)
                sh.append(h)

            # effective v bias as a row (per-free-column bias for V^T)
            beffr = {}
            for w in "v":
                rp_t = psum_t(f"br{w}")
                rp = rp_t[:1, :C]
                for i in range(2):
                    nc.tensor.matmul(rp, lhsT=sh[i], rhs=wT[w][i],
                                     start=(i == 0), stop=(i == 1))
                bt = ptile([1, C], f"beff{w}", F32R)
                nc.vector.tensor_add(out=bt, in0=rp, in1=brow[w])
                beffr[w] = bt
            # effective q,k biases as columns (per-partition bias for ACT fuse)
            beffc = {}
            for w in "qk":
                beffc[w] = []
                for j in range(2):
                    bp_t = psum_t(f"bc{w}{j}")
                    bp = bp_t[:128, :1]
                    for i in range(2):
                        nc.tensor.matmul(bp,
                                         lhsT=wT[w][i][:, 128 * j:128 * (j + 1)].bitcast(F32),
                                         rhs=sh[i].bitcast(F32),
                                         start=(i == 0), stop=(i == 1))
                    t = ptile([128, 1], f"beffc{w}{j}")
                    nc.vector.tensor_add(out=t, in0=bp, in1=bcol[w][j])
                    beffc[w].append(t)

            # fold GN scale into conv weights: q in place (f32r),
            # k/v as scaled bf16 copies (convs run bf16 against bf16 xf)
            for i in range(2):
                nc.vector.tensor_scalar_mul(out=wT["q"][i], in0=wT["q"][i],
                                            scalar1=sc[i])
            w_bf = {}
            for w in "kv":
                w_bf[w] = []
                for i in range(2):
                    t = ptile([128, C], f"w{w}bf{i}", BF16)
                    nc.scalar.activation(out=t, in_=wT[w][i].bitcast(F32),
                                         func=AF.Identity, scale=sc[i])
                    w_bf[w].append(t)
            ones_bf = ptile([1, 128], "ones_bf", BF16)
            nc.vector.memset(ones_bf, 1.0)
            beffr_bf = ptile([1, C], "beffr_bf", BF16)
            nc.vector.tensor_copy(out=beffr_bf, in_=beffr["v"].bitcast(F32))

            # ---------------- convs: K, Q (fp8 DoubleRow layout), V^T (bf16)
            # k8/q8 layout [ki, t, col]: channel c = t*128 + ki
            k8 = ptile([128, 2, N], "k8", FP8)
            q8 = ptile([128, 2, NH], "q8", FP8)
            for s2 in range(N // 1024):
                for j in range(2):
                    kp = psum_t(f"kp{j}_{s2}")
                    for half in range(2):
                        cols = slice(1024 * s2 + 512 * half,
                                     1024 * s2 + 512 * (half + 1))
                        for i in range(2):
                            nc.tensor.matmul(
                                kp[:, 512 * half:512 * (half + 1)],
                                lhsT=w_bf["k"][i][:, 128 * j:128 * (j + 1)],
                                rhs=xf[i][:, cols],
                                start=(i == 0), stop=(i == 1))
                    nc.scalar.activation(
                        out=k8[:, j, 1024 * s2:1024 * (s2 + 1)],
                        in_=kp, func=AF.Identity, bias=beffc["k"][j])
            for s2 in range(NH // 1024):
                for j in range(2):
                    qp = psum_t(f"qp{j}_{s2}")
                    for half in range(2):
                        cols = slice(1024 * s2 + 512 * half,
                                     1024 * s2 + 512 * (half + 1))
                        for i in range(2):
                            nc.tensor.matmul(
                                qp[:, 512 * half:512 * (half + 1)],
                                lhsT=_r(wT["q"][i][:, 128 * j:128 * (j + 1)]),
                                rhs=_r(xh[i][:, cols]),
                                start=(i == 0), stop=(i == 1))
                    nc.scalar.activation(
                        out=q8[:, j, 1024 * s2:1024 * (s2 + 1)],
                        in_=qp, func=AF.Identity, bias=beffc["q"][j])

            vT = [ptile([128, C], f"vT{rt}", BF16) for rt in range(N // 128)]

            def emit_vt(rt):
                # own PSUM tag: vp must not steal sp-pool slots during
                # produce(0); the o_ps banks are still free at that point
                vp_t = pop.tile([128, 512], F32, tag="o", name=f"vp{rt}")
                vp = vp_t[:, :C]
                for i in range(2):
                    nc.tensor.matmul(vp,
                                     lhsT=xf[i][:, 128 * rt:128 * (rt + 1)],
                                     rhs=w_bf["v"][i],
                                     start=(i == 0), stop=False)
                nc.tensor.matmul(vp, lhsT=ones_bf, rhs=beffr_bf,
                                 start=False, stop=True)
                t = vT[rt]
                if rt % 4 == 0:
                    nc.scalar.activation(out=t, in_=vp, func=AF.Identity)
                else:
                    nc.vector.tensor_copy(out=t, in_=vp)

            # bf16 copy of out-proj weights (O path runs bf16)
            wo_bf = []
            for i in range(2):
                t = ptile([128, C], f"wo_bf{i}", BF16)
                nc.vector.tensor_copy(out=t, in_=wT["o"][i].bitcast(F32))
                wo_bf.append(t)

            o_sb = [ptile([128, NH], "o_sb0", BF16), ptile([128, NH], "o_sb1", BF16)]

            kstage = os.environ.get("KSTAGE", "full")
            if kstage == "conv":
                for ct in range(2):
                    dbg = pout.tile([128, NH], F32, tag="dbg", name=f"dbg{ct}",
                                    bufs=2)
                    nc.vector.tensor_copy(out=dbg, in_=k8[:, ct, :NH])
                    nc.sync.dma_start(out=out_d[128 * ct:128 * (ct + 1), :],
                                      in_=dbg)

            # ---------------- attention (qt-phased, SW-pipelined) ----------
            # e_mega layout: idx = pair*1024 + t*512 + h*8 + ww
            # u_mega layout: idx = pair*512 + t*256 + ww*32 + h'
            # d/r layout:    idx = pair*16 + t*8 + ww
            NPAIR = N // 256     # 16 pairs of 128-key blocks
            NGP = 12             # pairs folded on GpSimd; rest on DVE
            NCH = 4              # pairs per reduce chunk
            e_ms = {}

            def emit_produce(qt):
                e_m = pe_pool.tile([128, NPAIR * 1024], BF16, tag="emega",
                                   name=f"em{qt}")
                e_ms[qt] = e_m
                for p in range(NPAIR):
                    emit_produce_pair(qt, p)

            def emit_produce_pair(qt, p):
                if p == 0 and qt not in e_ms:
                    e_ms[qt] = pe_pool.tile([128, NPAIR * 1024], BF16,
                                            tag="emega", name=f"em{qt}")
                e_m = e_ms[qt]
                qcols = slice(512 * qt, 512 * (qt + 1))
                sp = psum_t(f"sp{qt}_{p}")
                for t01 in range(2):
                    kb = 256 * p + 128 * t01
                    nc.tensor.matmul(
                        sp[:, 512 * t01:512 * (t01 + 1)],
                        lhsT=k8[:, :, kb:kb + 128],
                        rhs=q8[:, :, qcols],
                        start=True, stop=True, perf_mode=DR)
                # exp(S/16) -> bf16, one 2-bank ACT op
                nc.scalar.activation(out=e_m[:, 1024 * p:1024 * (p + 1)],
                                     in_=sp, func=AF.Exp, scale=1.0 / 16.0)

            o_pss = {}
            r_ms = {}

            def hfold(src, dst, hs, g, eng=None):
                # src [p, g, hs, 8] -> dst [p, g, hs/2, 8]  (packed-inner 2x)
                i0 = bass.AP(tensor=src.tensor, offset=src.offset,
                             ap=[src.ap[0], [hs * 8, g], [8, hs // 2], [1, 8]])
                i1 = bass.AP(tensor=src.tensor,
                             offset=src.offset + (hs // 2) * 8,
                             ap=[src.ap[0], [hs * 8, g], [8, hs // 2], [1, 8]])
                o = bass.AP(tensor=dst.tensor, offset=dst.offset,
                            ap=[dst.ap[0], [hs * 4, g], [8, hs // 2], [1, 8]])
                (eng or nc.vector).tensor_add(out=o, in0=i0, in1=i1)

            def emit_chain(qt, g):
                """softmax denominators for half g of qt (DVE fold chain)."""
                e_m = e_ms[qt]
                if g == 0:
                    o_pss[qt] = [pop.tile([128, 512], F32, tag="o",
                                          name=f"ops{qt}_{ct}")
                                 for ct in range(2)]
                if kstage == "attn_nonorm":
                    return
                HG = 16          # (pair, t) groups in a half
                e_h = e_m[:, 8192 * g:8192 * (g + 1)]
                u1 = pu_pool.tile([128, 4096], BF16, tag="u1",
                                  name=f"u1_{qt}_{g}")
                u2 = pu_pool.tile([128, 2048], BF16, tag="u2",
                                  name=f"u2_{qt}_{g}")
                d_m = pd_pool.tile([128, 128], F32, tag="d", name=f"d{qt}_{g}")
                r_m = pd_pool.tile([128, 128], BF16, tag="r", name=f"r{qt}_{g}")
                r_ms[(qt, g)] = r_m
                hfold(e_h, u1, 64, HG)                 # h 64 -> 32
                hfold(u1, u2, 32, HG)                  # 32 -> 16
                hfold(u2, u1[:, :1024], 16, HG)        # 16 -> 8
                hfold(u1[:, :1024], u2[:, :512], 8, HG)  # 8 -> 4
                hfold(u2[:, :512], u1[:, 1024:1280], 4, HG)  # 4 -> 2
                u5 = u1[:, 1024:1280]
                i0 = bass.AP(tensor=u5.tensor, offset=u5.offset,
                             ap=[u5.ap[0], [16, HG], [1, 8]])
                i1 = bass.AP(tensor=u5.tensor, offset=u5.offset + 8,
                             ap=[u5.ap[0], [16, HG], [1, 8]])
                nc.vector.tensor_add(out=d_m, in0=i0, in1=i1)
                nc.vector.reciprocal(out=r_m, in_=d_m)

            def emit_consume_pair(qt, p):
                e_m = e_ms[qt]
                o_ps = o_pss[qt]
                p0 = 8 * (p // 8)
                if kstage != "attn_nonorm":
                    r_m = r_ms[(qt, p // 8)]
                    # attn = e*r (bcast over h via middle axis; 2x_1p)
                    e4 = bass.AP(tensor=e_m.tensor,
                                 offset=e_m.offset + 1024 * p,
                                 ap=[e_m.ap[0], [512, 2], [8, 64], [1, 8]])
                    r4 = bass.AP(tensor=r_m.tensor,
                                 offset=r_m.offset + 16 * (p - p0),
                                 ap=[r_m.ap[0], [8, 2], [0, 64], [1, 8]])
                    nc.vector.tensor_mul(out=e4, in0=e4, in1=r4)
                for ct in range(2):
                    for t01 in range(2):
                        nc.tensor.matmul(
                            o_ps[ct],
                            lhsT=vT[2 * p + t01][:, 128 * ct:128 * (ct + 1)],
                            rhs=e_m[:, 1024 * p + 512 * t01:
                                    1024 * p + 512 * (t01 + 1)],
                            start=(p == 0 and t01 == 0),
                            stop=(p == NPAIR - 1 and t01 == 1))

            def emit_finish(qt):
                o_ps = o_pss.pop(qt)
                del e_ms[qt]
                r_ms.pop((qt, 0), None)
                r_ms.pop((qt, 1), None)
                qcols = slice(512 * qt, 512 * (qt + 1))
                for ct in range(2):
                    nc.scalar.activation(out=o_sb[ct][:, qcols], in_=o_ps[ct],
                                         func=AF.Identity)
                # out-proj + residual for this quarter
                for ct in range(2):
                    prj_t = psum_t(f"prj{qt}_{ct}")
                    prj = prj_t[:, :512]
                    for i in range(2):
                        nc.tensor.matmul(
                            prj,
                            lhsT=wo_bf[i][:, 128 * ct:128 * (ct + 1)],
                            rhs=o_sb[i][:, qcols],
                            start=(i == 0), stop=(i == 1))
                    ot = pout.tile([128, 512], F32, tag="ot",
                                   name=f"ot{qt}_{ct}")
                    nc.vector.scalar_tensor_tensor(
                        out=ot, in0=prj, scalar=bcol["o"][ct],
                        in1=xh[ct][:, qcols].bitcast(F32),
                        op0=ALU.add, op1=ALU.add)
                    nc.sync.dma_start(out=out_d[128 * ct:128 * (ct + 1), qcols],
                                      in_=ot)

            if kstage == "conv":
                for rt in range(N // 128):
                    emit_vt(rt)
            if kstage != "conv":
                # pair-granular software pipeline: S-MMs of qt interleave
                # with O-MMs of qt-1 so exp-throttled S never idles the PE;
                # the V^T conv fills the PE during the first produce phase
                for p in range(NPAIR):
                    emit_produce_pair(0, p)
                    emit_vt(2 * p)
                    emit_vt(2 * p + 1)
                for qt in range(1, 4):
                    emit_chain(qt - 1, 0)
                    for p in range(NPAIR):
                        emit_produce_pair(qt, p)
                        if p == 8:
                            emit_chain(qt - 1, 1)
                        if p >= 2:
                            emit_consume_pair(qt - 1, p - 2)
                    for p in range(NPAIR - 2, NPAIR):
                        emit_consume_pair(qt - 1, p)
                    emit_finish(qt - 1)
                emit_chain(3, 0)
                emit_chain(3, 1)
                for p in range(NPAIR):
                    emit_consume_pair(3, p)
                emit_finish(3)
    nc.compile()
    return nc


_NC = None


def _get_nc():
    global _NC
    if _NC is None:
        _NC = build_nc()
    return _NC


def _prep_in_maps(x, gamma, beta, q_w, q_b, k_w, k_b, v_w, v_b, o_w, o_b):
    x = np.ascontiguousarray(np.asarray(x, np.float32))
    g1 = np.zeros((C, GROUPS), np.float32)
    g1[np.arange(C), np.arange(C) // (C // GROUPS)] = 1.0
    wall = np.concatenate(
        [np.asarray(w, np.float32).T for w in (q_w, k_w, v_w, o_w)],
        axis=1)
    pcols = np.zeros((C, 40), np.float32)
    pcols[:, 0] = np.asarray(gamma, np.float32)
    pcols[:, 1] = np.asarray(beta, np.float32)
    pcols[:, 2] = np.asarray(q_b, np.float32)
    pcols[:, 3] = np.asarray(k_b, np.float32)
    pcols[:, 4] = np.asarray(o_b, np.float32)
    pcols[:, 5:37] = g1
    shared = {
        "wall": np.ascontiguousarray(wall),
        "pcols": pcols,
        "smalls": np.ascontiguousarray(g1.T),
        "bv_row": np.asarray(v_b, np.float32).reshape(1, C).copy(),
    }
    in_maps = []
    import ml_dtypes
    for core in range(8):
        b, half = core // 2, core % 2
        xb = np.ascontiguousarray(x[b].reshape(C, N)).astype(ml_dtypes.bfloat16)
        # queries h-major inside each 512 block: q = qt*512 + h*8 + ww
        xh = x[b][:, :, half * WH:(half + 1) * WH]           # [C, 64h, 32w']
        xh = np.ascontiguousarray(
            xh.reshape(C, H, 4, 8).transpose(0, 2, 1, 3)
        ).reshape(C, NH)
        in_maps.append(dict(shared, xf=np.ascontiguousarray(xb), xh=xh))
    return in_maps


def run(trace=False, **inputs):
    in_maps = _prep_in_maps(**inputs)
    nc = _get_nc()
    res = run_bass_kernel_spmd(nc, in_maps, core_ids=list(range(8)), trace=trace)
    x = np.asarray(inputs["x"], np.float32)
    out = np.empty((B, C, H, W), np.float32)
    for core in range(8):
        b, half = core // 2, core % 2
        od = res.results[core]["out"]                        # [C, 2048]
        oh = od.reshape(C, 4, H, 8).transpose(0, 2, 1, 3).reshape(C, H, WH)
        out[b][:, :, half * WH:(half + 1) * WH] = oh
    return out, res


def kernel(**inputs):
    out, _ = run(trace=False, **inputs)
    return out
